# revision 37
# baseline (speedup 1.0000x reference)
"""Batched spline reconstruction (B-spline / NURBS / Bezier curves) on 8 TRN2
NeuronCores.

Math (per batch element b, coordinate d, sample point n):
    bspline[b,d,n] = sum_i basis[i,n]  * bspline_cp[b,i,d]
    bezier [b,d,n] = sum_i bernT[i,n]  * bezier_cp[b,i,d]
    nurbs  [b,d,n] = (sum_i w[b,i]*basis[i,n]*nurbs_cp[b,i,d])
                     / (sum_i w[b,i]*basis[i,n] + 1e-8)

The basis matrices ([n_cp, num_points], batch-independent) are computed
host-side and replicated to every core.  Batch is sharded 8 ways (pure data
parallel).  Per core everything is a K=32 contraction on the TensorEngine:

    out[(b,d), n] = lhsT[k, (b,d)].T @ rhs[k, n]

with lhsT = control points transposed host-side to [n_cp, B_loc*2].

The whole pipeline runs in bf16 (except PSUM accumulation and the NURBS
reciprocal, which stay f32): inputs are cast to bf16 host-side, the device
writes bf16 outputs, and the host upcasts to f32 on gather.  Measured end to
end rel err ~7e-3 vs the f32 reference - an order of magnitude under the 2e-2
gate - while halving both the HBM write traffic (the roofline term: 6 MB/core
instead of 12 MB) and the PE streaming time.

The four K=32 matmuls per chunk (bspline / bezier / NURBS-numerator /
NURBS-denominator) are packed into the four 32-row groups of the PE array via
tile_position so they execute concurrently.  Post-PSUM work is split across
the two engines with PSUM ports: ScalarE copies bsp/bez (f32 PSUM -> bf16
SBUF), the DVE does the NURBS reciprocal (f32) + multiply (-> bf16); these
copies are the pipeline pacer (~0.6 us per [128, 512] op, f32-PSUM sources
are capped at 1x accel on TRN2).

Two structural tricks reduce the DVE's share: (1) the NURBS numerator is
computed d-MAJOR - each 128-partition block holds one coordinate component
for 128 DISTINCT batch elements - so a single denominator matmul +
reciprocal serves two numerator blocks (the denominator has no d
dependence), halving the reciprocal work; (2) w_eps (epsilon folded in) is
multiplied into the control points on the host, so no weight broadcasting
or multiplication happens on-device at all.

Stores are 256 KB half-block DMAs of contiguous bf16 rows, with the final
block stored in 128 KB pieces to shorten the end-of-kernel drain.  bsp/bez
stores ride the sync HWDGE queue (trigger slots there are ~0.6 us each - a
scarce resource); the NURBS stores ride the otherwise-idle GpSimd (SWDGE)
queue so the tail triggers issue on two queues in parallel.  A fixed ~7 us
NEFF postamble (251 serial semaphore clears, Tensor-queue paced) and ~2 us
DMA completion latency bound what the tail can reach.
"""

import numpy as np

B = 2048          # total batch
NCP = 32          # control points per curve
NPT = 2048        # num_points
NCORES = 8
BLOC = B // NCORES          # 256 batch elements per core
ROWS = BLOC * 2             # 512 (b,d) rows per core
P = 128                     # partition block
NBLK = ROWS // P            # 4 row blocks
NFREE = 512                 # matmul moving free dim (1 PSUM bank of f32)
NCH = NPT // NFREE          # 4 column chunks
DEGREE = 3
EPS = 1e-8

_CACHE = {}


def _bf16():
    import ml_dtypes

    return ml_dtypes.bfloat16


# ---------------------------------------------------------------- host math
def _basis_matrices():
    """Static [4*NCP, NPT] stacked moving operands in bf16:
    [basis; bernstein; basis; basis] for PE row groups g0..g3."""
    p = DEGREE
    internal = np.linspace(0.0, 1.0, NCP - p + 1)[1:-1]
    knots = np.concatenate([np.zeros(p + 1), internal, np.ones(p + 1)])
    t = np.linspace(knots[p], knots[-p - 1], NPT)

    left = knots[:NCP]
    right = knots[1:NCP + 1]
    N = ((t[None, :] >= left[:, None]) & (t[None, :] < right[:, None])).astype(
        np.float64
    )
    N[-1] = ((t >= left[-1]) & (t <= right[-1])).astype(np.float64)
    for d in range(1, p + 1):
        d1 = knots[d:d + NCP] - knots[:NCP]
        d2 = knots[d + 1:d + 1 + NCP] - knots[1:1 + NCP]
        s1 = np.where(d1 != 0, d1, 1.0)
        s2 = np.where(d2 != 0, d2, 1.0)
        term1 = np.where(
            d1[:, None] != 0,
            (t[None, :] - knots[:NCP, None]) / s1[:, None] * N,
            0.0,
        )
        N_shift = np.concatenate([N[1:], np.zeros((1, N.shape[1]))], axis=0)
        term2 = np.where(
            d2[:, None] != 0,
            (knots[d + 1:d + 1 + NCP, None] - t[None, :]) / s2[:, None] * N_shift,
            0.0,
        )
        N = term1 + term2
    basis = N  # float64 [NCP, NPT]

    # Bernstein basis, transposed to [NCP, NPT].  Exact binomials in f64;
    # bf16 rounding (~4e-3) swamps the reference's f32 gammaln error (~6e-4),
    # so no need to replicate the device gammaln here.
    from math import comb

    nb = NCP - 1
    tb = np.linspace(0.0, 1.0, NPT)
    i = np.arange(nb + 1)
    coeff = np.array([comb(nb, k) for k in i], dtype=np.float64)
    bernT = (
        coeff[:, None]
        * tb[None, :] ** i[:, None]
        * (1.0 - tb[None, :]) ** (nb - i)[:, None]
    )

    basis_rep = np.concatenate([basis, bernT, basis, basis], axis=0)
    return np.ascontiguousarray(basis_rep.astype(_bf16()))


# ---------------------------------------------------------------- device IR
def _build_nc():
    import concourse.bass as bass
    import concourse.tile as tile
    from concourse import bacc, mybir

    f32 = mybir.dt.float32
    bf = mybir.dt.bfloat16

    nc = bacc.Bacc("TRN2", target_bir_lowering=False, debug=False)

    basis_d = nc.dram_tensor("basis_rep", [P, NPT], bf, kind="ExternalInput")
    in2_d = nc.dram_tensor("in2", [P, ROWS + BLOC], bf, kind="ExternalInput")
    obsp_d = nc.dram_tensor("out_bsp", [BLOC, 2, NPT], bf, kind="ExternalOutput")
    onur_d = nc.dram_tensor("out_nur", [BLOC, 2, NPT], bf, kind="ExternalOutput")
    obez_d = nc.dram_tensor("out_bez", [BLOC, 2, NPT], bf, kind="ExternalOutput")

    obsp_v = obsp_d[:].rearrange("b d n -> (b d) n")
    obez_v = obez_d[:].rearrange("b d n -> (b d) n")
    # NURBS output is produced in d-major layout (see below)
    onur_dm = onur_d[:].rearrange("b d n -> d b n")

    G0, G1, G2, G3 = 0, 32, 64, 96  # PE row groups: bsp, bez, num, den

    with tile.TileContext(nc) as tc:
        with (
            tc.tile_pool(name="const", bufs=1) as cpool,
            tc.tile_pool(name="outp", bufs=3) as opool,
            tc.tile_pool(name="aux", bufs=4) as apool,
            tc.tile_pool(name="psum", bufs=2, space=bass.MemorySpace.PSUM) as ppool,
        ):
            # one tile per basis column chunk so the first matmul only waits
            # on the first chunk's DMA, not all four
            basis_t = [
                cpool.tile([P, NFREE], bf, name=f"basis{i}", tag=f"basis{i}")
                for i in range(NCH)
            ]
            stack_s = cpool.tile([P, ROWS + BLOC], bf, tag="stack")

            # bsp/bez stationary rows and basis chunk 0 gate the first
            # matmul; the rest of in2 (NURBS cp + weights) follows
            nc.sync.dma_start(stack_s[:G2, :ROWS], in2_d[:G2, :ROWS])
            nc.sync.dma_start(basis_t[0][:], basis_d[:, 0:NFREE])
            nc.sync.dma_start(stack_s[G2:, :], in2_d[G2:, :])
            for nch in range(1, NCH):
                sl = slice(nch * NFREE, (nch + 1) * NFREE)
                nc.sync.dma_start(basis_t[nch][:], basis_d[:, sl])

            # NURBS layouts: the numerator is computed d-MAJOR - slot s of
            # the g2 loop covers (pair, d) = (s//2, s%2) with 128 DISTINCT
            # batch elements as PSUM partitions - so ONE denominator matmul
            # + reciprocal per PAIR of slots serves both d components
            # (halves the DVE reciprocal work; den has no d dependence).
            # The g2 stationary arrives from the host with w_eps already
            # folded in (f32 multiply host-side - no on-device weight prep),
            # laid out (pair, d, b)-major; g3 gets the raw wT slab.

            recs = {}
            for blk in range(NBLK):
                pair, dd = blk // 2, blk % 2
                cols = slice(blk * P, (blk + 1) * P)
                ob = opool.tile([P, NPT], bf, tag="ob")
                on = opool.tile([P, NPT], bf, tag="on")
                oz = opool.tile([P, NPT], bf, tag="oz")
                rows = slice(blk * P, (blk + 1) * P)
                nrows = slice(pair * P, (pair + 1) * P)
                last_blk = blk == NBLK - 1
                for nch in range(NCH):
                    sl = slice(nch * NFREE, (nch + 1) * NFREE)
                    ps_b = ppool.tile([P, NFREE], f32, tag="psb")
                    ps_z = ppool.tile([P, NFREE], f32, tag="psz")
                    bs = basis_t[nch]
                    nc.tensor.matmul(
                        ps_b[:], stack_s[:G1, cols], bs[:G1, :],
                        start=True, stop=True, tile_position=(G0, 0),
                    )
                    nc.tensor.matmul(
                        ps_z[:], stack_s[G1:G2, cols], bs[G1:G2, :],
                        start=True, stop=True, tile_position=(G1, 0),
                    )
                    if dd == 0:
                        ps_d = ppool.tile([P, NFREE], f32, tag="psd")
                        nc.tensor.matmul(
                            ps_d[:],
                            stack_s[G3:, ROWS + pair * P:ROWS + (pair + 1) * P],
                            bs[G3:, :],
                            start=True, stop=True, tile_position=(G3, 0),
                        )
                    ps_n = ppool.tile([P, NFREE], f32, tag="psn")
                    nc.tensor.matmul(
                        ps_n[:], stack_s[G2:G3, cols], bs[G2:G3, :],
                        start=True, stop=True, tile_position=(G2, 0),
                    )
                    nc.scalar.copy(ob[:, sl], ps_b[:])
                    if dd == 1 and nch == 2:
                        # odd blocks run no reciprocal - the DVE has slack
                        # there, so it takes one bez copy for engine balance
                        nc.vector.tensor_copy(oz[:, sl], ps_z[:])
                    else:
                        nc.scalar.copy(oz[:, sl], ps_z[:])
                    if dd == 0:
                        rec = apool.tile([P, NFREE], f32, tag="rec")
                        recs[(pair, nch)] = rec
                        nc.vector.reciprocal_approx_fast(out=rec[:], in_=ps_d[:])
                    else:
                        rec = recs[(pair, nch)]
                    nc.vector.tensor_mul(on[:, sl], ps_n[:], rec[:])
                    if last_blk:
                        # final block: half-block store at nch 1, then 128 KB
                        # pieces so the post-compute drain is short; nur
                        # first (the DVE finishes before the ScalarE)
                        if nch == 1:
                            hl = slice(0, 2 * NFREE)
                            nc.gpsimd.dma_start(onur_dm[dd, nrows, hl], on[:, hl])
                            nc.sync.dma_start(obsp_v[rows, hl], ob[:, hl])
                            nc.sync.dma_start(obez_v[rows, hl], oz[:, hl])
                        elif nch >= 2:
                            # tail triggers fan out over three queues: nur
                            # on gpsimd, bsp on sync, bez on the scalar
                            # HWDGE ring.  Both bez triggers are emitted at
                            # nch 3 so they sit AFTER the final copies in
                            # the ScalarE queue order (a trigger placed
                            # between copies would stall the copy stream).
                            nc.gpsimd.dma_start(onur_dm[dd, nrows, sl], on[:, sl])
                            nc.sync.dma_start(obsp_v[rows, sl], ob[:, sl])
                            if nch == NCH - 1:
                                s2 = slice(2 * NFREE, 3 * NFREE)
                                nc.scalar.dma_start(obez_v[rows, s2], oz[:, s2])
                                nc.scalar.dma_start(obez_v[rows, sl], oz[:, sl])
                    elif nch % 2 == 1:
                        hl = slice((nch - 1) * NFREE, (nch + 1) * NFREE)
                        nc.sync.dma_start(obsp_v[rows, hl], ob[:, hl])
                        nc.sync.dma_start(obez_v[rows, hl], oz[:, hl])
                        nc.gpsimd.dma_start(onur_dm[dd, nrows, hl], on[:, hl])

    nc.compile()
    return nc


def _get_state():
    if "nc" not in _CACHE:
        _CACHE["nc"] = _build_nc()
        _CACHE["basis_rep"] = _basis_matrices()
    return _CACHE["nc"], _CACHE["basis_rep"]


def _prep_in_maps(bspline_cp, nurbs_cp, nurbs_weights, bezier_cp, basis_rep):
    bf16 = _bf16()
    bspline_cp = np.asarray(bspline_cp, dtype=np.float32)
    nurbs_cp = np.asarray(nurbs_cp, dtype=np.float32)
    bezier_cp = np.asarray(bezier_cp, dtype=np.float32)
    # fold the NURBS epsilon into the weights: basis rows sum to 1, so
    # sum_i (w_i+eps)*N_i == sum_i w_i*N_i + eps exactly
    w_eps = (np.asarray(nurbs_weights, np.float64) + EPS).astype(np.float32)

    in_maps = []
    for c in range(NCORES):
        sl = slice(c * BLOC, (c + 1) * BLOC)
        in2 = np.zeros((P, ROWS + BLOC), bf16)
        in2[0:32, :ROWS] = (
            bspline_cp[sl].transpose(1, 0, 2).reshape(NCP, ROWS)
        )
        in2[32:64, :ROWS] = (
            bezier_cp[sl].transpose(1, 0, 2).reshape(NCP, ROWS)
        )
        # weighted NURBS control points (w_eps folded in host-side, f32)
        # in d-MAJOR slot order: column index is (pair, d, b) so slot
        # s = (s//2, s%2) covers 128 distinct b
        in2[64:96, :ROWS] = (
            (w_eps[sl][:, :, None] * nurbs_cp[sl])
            .reshape(NBLK // 2, P, NCP, 2)     # [pair, b, i, d]
            .transpose(2, 0, 3, 1)             # [i, pair, d, b]
            .reshape(NCP, ROWS)
        )
        wT = w_eps[sl].T  # [NCP, BLOC], b-major (no d broadcast)
        in2[96:128, ROWS:] = wT
        in_maps.append({"basis_rep": basis_rep, "in2": in2})
    return in_maps


# ---------------------------------------------------------------- entry point
def kernel(bspline_cp, nurbs_cp, nurbs_weights, bezier_cp, num_points,
           _trace=False):
    assert int(num_points) == NPT, f"kernel compiled for num_points={NPT}"
    from concourse.bass_utils import run_bass_kernel_spmd

    nc, basis_rep = _get_state()
    in_maps = _prep_in_maps(
        bspline_cp, nurbs_cp, nurbs_weights, bezier_cp, basis_rep
    )

    # the device occasionally reports NRT_EXEC_UNIT_UNRECOVERABLE transiently
    # (clears on reopen); retry a few times before giving up
    last_exc = None
    for attempt in range(3):
        try:
            res = run_bass_kernel_spmd(
                nc, in_maps, list(range(NCORES)), trace=_trace
            )
            break
        except Exception as e:
            last_exc = e
            import time

            time.sleep(3.0)
    else:
        raise last_exc
    kernel.last_results = res

    def gather(name):
        return np.concatenate(
            [np.asarray(res.results[c][name]) for c in range(NCORES)], axis=0
        ).astype(np.float32)

    return gather("out_bsp"), gather("out_nur"), gather("out_bez")
